# revision 11
# baseline (speedup 1.0000x reference)
"""Ball attention (block-local attention, ball size 128) on 8 Trainium2 cores.

Reference computation (per (b,h) head, per ball of 128 consecutive tokens):
    S = Q K^T / sqrt(64);  P = softmax(S, axis=-1);  O = P V

Sharding: the 64 (b,h) heads are split 8-per-core (pure data parallel).

Per-core design (all sizes measured on HW):
  * Loads/stores use the natural ball layout (seq position on partitions).
  * Q^T/K^T via packed 2-ball PE transposes: input [128 seq, 2ball x 64d]
    -> output [2ball x 64d partitions, 128 seq]; ball b of a pair lands on
    partition half 64b. ~173ns per transpose op (PE SBUF latency bound).
  * PSUM->SBUF copies round to float32r (DVE for Q^T, ACT for K^T).
  * S^T = K Q^T as float32r "junk-pair" matmuls: rhs = [qt(ball) | qt(ball+2)]
    gives N=256 which runs at 1 cyc/row (vs 4 for fp32); half the output is
    discarded. Measured 274ns/ball, rel err ~1.7e-4 on S (float32r rounds
    mantissas; final output error ~3e-5).
  * E = exp(S^T/8) on ACT directly into SBUF; the ones-column appended to V
    provides softmax denominators through the O matmul; normalize on DVE via
    a per-partition reciprocal broadcast.
  * O matmul dtype selectable (BALL_O_DTYPE): bf16 (fast, adds ~1e-3 error),
    float32r, or float32.
"""

import os
import sys

for _p in ("/opt/trn_rl_repo",):
    if _p not in sys.path and os.path.isdir(_p):
        sys.path.insert(0, _p)

from contextlib import ExitStack

import numpy as np

import concourse.bass as bass
import concourse.mybir as mybir
import concourse.tile as tile
from concourse import bacc
from concourse._compat import with_exitstack
from concourse.masks import make_identity

B, H, N, DH = 4, 16, 8192, 64
BS = 128                 # ball size == SBUF partition count
NCORES = 8
HEADS = B * H // NCORES  # heads per core (8)
M = N // BS              # balls per head (64)

FP32 = mybir.dt.float32
FP32R = mybir.dt.float32r
BF16 = mybir.dt.bfloat16

S_MODE = os.environ.get("BALL_S_MODE", "fp32r")   # fp32r | fp32
O_MODE = os.environ.get("BALL_O_MODE", "bf16")    # bf16 | fp32r | fp32
GRP = 4
# debug bisect: 1=transposes+copies, 2=+S+exp, 3=+O, 4=full (default)
STAGE = int(os.environ.get("BALL_STAGE", "4"))


@with_exitstack
def ball_attention_kernel(
    ctx: ExitStack,
    tc: tile.TileContext,
    out_ap: bass.AP,
    q_ap: bass.AP,
    k_ap: bass.AP,
    v_ap: bass.AP,
    heads: int = HEADS,
    m: int = M,
):
    nc = tc.nc
    assert m % GRP == 0
    ngrp = m // GRP
    scale = 1.0 / float(np.sqrt(DH))
    t_dt = FP32R if S_MODE == "fp32r" else FP32
    if O_MODE == "bf16":
        e_dt = v_dt = BF16
    elif O_MODE == "fp32r":
        e_dt = v_dt = FP32R
    else:
        e_dt = v_dt = FP32

    const_pool = ctx.enter_context(tc.tile_pool(name="const", bufs=1))
    io_pool = ctx.enter_context(tc.tile_pool(name="io", bufs=2))
    t_sb_pool = ctx.enter_context(tc.tile_pool(name="t_sb", bufs=3))
    e_pool = ctx.enter_context(tc.tile_pool(name="e", bufs=2))
    r_pool = ctx.enter_context(tc.tile_pool(name="r", bufs=2))
    t_ps_pool = ctx.enter_context(tc.tile_pool(name="t_ps", bufs=2, space="PSUM"))
    s_ps_pool = ctx.enter_context(tc.tile_pool(name="s_ps", bufs=2, space="PSUM"))
    o_ps_pool = ctx.enter_context(tc.tile_pool(name="o_ps", bufs=2, space="PSUM"))

    ident = const_pool.tile([BS, BS], FP32)
    make_identity(nc, ident)

    for h in range(heads):
        # ---- loads (natural ball layout: partition = seq within ball) -----
        q_sb = io_pool.tile([BS, m, DH], FP32, tag="q")
        nc.sync.dma_start(q_sb, q_ap[h].rearrange("(mm p) d -> p mm d", p=BS))
        k_sb = io_pool.tile([BS, m, DH], FP32, tag="k")
        nc.sync.dma_start(k_sb, k_ap[h].rearrange("(mm p) d -> p mm d", p=BS))
        vt = io_pool.tile([BS, m, DH + 1], v_dt, tag="vt")
        if v_dt == FP32:
            nc.sync.dma_start(vt[:, :, 0:DH], v_ap[h].rearrange("(mm p) d -> p mm d", p=BS))
        else:  # SWDGE cast during DMA
            nc.gpsimd.dma_start(vt[:, :, 0:DH], v_ap[h].rearrange("(mm p) d -> p mm d", p=BS))
        nc.vector.memset(vt[:, :, DH], 1.0)
        ob = io_pool.tile([BS, m, DH], FP32, tag="ob")

        for g in range(ngrp):
            # 4 balls: m0..m0+3; junk-pairs (m0, m0+2) and (m0+1, m0+3)
            m0 = g * GRP
            # one bank: [qt(pair0) | kt(pair0) | qt(pair1) | kt(pair1)]
            t_ps = t_ps_pool.tile([BS, 4, BS], FP32, tag="t")
            qt = t_sb_pool.tile([BS, 2, BS], t_dt, tag="qt")        # [pair, seq]
            kt = t_sb_pool.tile([BS, 2, BS], t_dt, tag="kt")
            # packed transposes: 2 balls per op; ball parity b -> partitions 64b
            nc.tensor.transpose(t_ps[:, 0, :], q_sb[:, m0 : m0 + 2, :], ident)
            nc.tensor.transpose(t_ps[:, 1, :], k_sb[:, m0 : m0 + 2, :], ident)
            nc.tensor.transpose(t_ps[:, 2, :], q_sb[:, m0 + 2 : m0 + 4, :], ident)
            nc.tensor.transpose(t_ps[:, 3, :], k_sb[:, m0 + 2 : m0 + 4, :], ident)
            # PSUM -> SBUF (+ round to float32r): DVE takes Q^T, ACT takes K^T
            nc.vector.tensor_copy(qt, t_ps[:, 0:4:2, :])
            nc.scalar.copy(kt, t_ps[:, 1:4:2, :])
            if STAGE == 1:
                nc.vector.tensor_copy(
                    ob[:, m0 : m0 + 2, :], qt[:, :, 0:DH].bitcast(FP32)
                )
                nc.vector.tensor_copy(
                    ob[:, m0 + 2 : m0 + 4, :], kt[:, :, 0:DH].bitcast(FP32)
                )
                continue

            # S^T matmuls. qt slot layout: [pair a' = 0|1][seq], ball (2j+b)
            # at partitions 64b. junk-pair rhs = qt[64b:64b+64, :, :] (N=256).
            # Consecutive matmuls must hit different PSUM banks (same-bank
            # back-to-back PE matmul writes fault): ball j -> bank j%2,
            # slot j//2 of a 2-bank tile.
            s_ps = s_ps_pool.tile([BS, 2, 2, 2 * BS], FP32, tag="s")
            e_sb = e_pool.tile([BS, GRP, BS], e_dt, tag="e")
            for j in range(GRP):
                a2, b = j >> 1, j & 1          # ball m0+j = pair a2, parity b
                lo = 64 * b
                if S_MODE == "fp32r":
                    nc.tensor.matmul(
                        s_ps[:, j % 2, j // 2, :],
                        kt[lo : lo + 64, a2, :],
                        qt[lo : lo + 64, :, :],
                        start=True,
                        stop=True,
                    )
                else:
                    nc.tensor.matmul(
                        s_ps[:, j % 2, j // 2, a2 * BS : a2 * BS + BS],
                        kt[lo : lo + 64, a2, :],
                        qt[lo : lo + 64, a2, :],
                        start=True,
                        stop=True,
                    )
            if STAGE == 15:
                for a2 in range(2):
                    nc.vector.tensor_copy(
                        ob[:, m0 + a2 * 2 : m0 + a2 * 2 + 2, :],
                        s_ps[:, :, a2, a2 * BS : a2 * BS + DH],
                    )
                continue
            # E = exp(S^T/8); good half of ball j's junk-pair output is the
            # column block of its own pair slot (a2 = j>>1 = tile slot dim).
            for a2 in range(2):
                nc.scalar.activation(
                    e_sb[:, a2 * 2 : a2 * 2 + 2, :],
                    s_ps[:, :, a2, a2 * BS : a2 * BS + BS],
                    mybir.ActivationFunctionType.Exp,
                    scale=scale,
                )

            if STAGE == 2:
                if O_MODE == "bf16":
                    for j in range(GRP):
                        nc.vector.tensor_copy(ob[:, m0 + j, :], e_sb[:, j, 0:DH])
                else:
                    nc.vector.tensor_copy(ob[:, m0 : m0 + GRP, :], e_sb[:, :, 0:DH].bitcast(FP32))
                continue

            # O_unnorm = E^T @ [V | 1]
            o_ps = o_ps_pool.tile([BS, GRP, DH + 1], FP32, tag="o")
            for j in range(GRP):
                nc.tensor.matmul(
                    o_ps[:, j, :],
                    e_sb[:, j, :],
                    vt[:, m0 + j, :],
                    start=True,
                    stop=True,
                )
            if STAGE == 3:
                nc.vector.tensor_copy(ob[:, m0 : m0 + GRP, :], o_ps[:, :, 0:DH])
                continue
            # normalize by the ones-column sums
            r_sb = r_pool.tile([BS, GRP], FP32, tag="r")
            nc.vector.reciprocal(r_sb, o_ps[:, :, DH])
            nc.vector.tensor_mul(
                ob[:, m0 : m0 + GRP, :],
                o_ps[:, :, 0:DH],
                r_sb.unsqueeze(2).broadcast_to([BS, GRP, DH]),
            )

        # ---- store -------------------------------------------------------
        nc.gpsimd.dma_start(out_ap[h].rearrange("(mm p) d -> p mm d", p=BS), ob)


def build_nc(heads: int = HEADS, m: int = M):
    nc = bacc.Bacc("TRN2", target_bir_lowering=False, debug=False, num_devices=NCORES)
    q = nc.dram_tensor("q", [heads, m * BS, DH], FP32, kind="ExternalInput").ap()
    k = nc.dram_tensor("k", [heads, m * BS, DH], FP32, kind="ExternalInput").ap()
    v = nc.dram_tensor("v", [heads, m * BS, DH], FP32, kind="ExternalInput").ap()
    o = nc.dram_tensor("out", [heads, m * BS, DH], FP32, kind="ExternalOutput").ap()
    with tile.TileContext(nc) as tc:
        ball_attention_kernel(tc, o, q, k, v, heads=heads, m=m)
    nc.compile()
    return nc


_NC_CACHE = {}


def kernel(q: np.ndarray, k: np.ndarray, v: np.ndarray) -> np.ndarray:
    from concourse.bass_utils import run_bass_kernel_spmd

    assert q.shape == (B, H, N, DH)
    if "nc" not in _NC_CACHE:
        _NC_CACHE["nc"] = build_nc()
    nc = _NC_CACHE["nc"]

    hpc = HEADS
    qf = np.ascontiguousarray(np.asarray(q, dtype=np.float32).reshape(B * H, N, DH))
    kf = np.ascontiguousarray(np.asarray(k, dtype=np.float32).reshape(B * H, N, DH))
    vf = np.ascontiguousarray(np.asarray(v, dtype=np.float32).reshape(B * H, N, DH))
    in_maps = [
        {
            "q": np.ascontiguousarray(qf[c * hpc : (c + 1) * hpc]),
            "k": np.ascontiguousarray(kf[c * hpc : (c + 1) * hpc]),
            "v": np.ascontiguousarray(vf[c * hpc : (c + 1) * hpc]),
        }
        for c in range(NCORES)
    ]
    res = run_bass_kernel_spmd(nc, in_maps, core_ids=list(range(NCORES)))
    out = np.concatenate([res.results[c]["out"] for c in range(NCORES)], axis=0)
    return out.reshape(B, H, N, DH)
